# revision 11
# baseline (speedup 1.0000x reference)
"""Multi-head attention (B=2, S=4096, D=768, H=12, HD=64) on 8 TRN2 NeuronCores.

Sharding: core c handles batch b = c//4 and heads [3*(c%4), 3*(c%4)+3).
Each core computes its 3 heads' attention plus the partial output
projection (row-split Wo); the host sums the 4 partials per batch.

Per-core kernel (Tile framework):
  stage A: X -> X^T via PE transposes; Q^T/K^T/V projections.
           Heads 0,1 are stacked into one 128-partition tile (head0 in
           partitions 0:64, head1 in 64:128) so their score matmuls
           row-pack and their projection matmuls col-pack on the PE.
  stage B: scores^T = K^T_tile.T @ Q^T chunk ([s_k=128, s_q=512] tiles,
           so softmax's sum runs along PSUM partitions); exp on ScalarE
           over [128, 1024] slabs with the 1/sqrt(HD) scale folded in;
           ctx^T accumulated with a ones-augmented V ([s_k, 65]) so
           row 64 of the ctx accumulator is the softmax denominator.
  stage C: normalize (DVE reciprocal + GpSimd partition broadcast),
           output projection (3 heads + bias row into one PSUM group).
"""

import os
import sys

import numpy as np


def _ensure_concourse():
    try:
        import concourse.bass  # noqa: F401
        return
    except ImportError:
        pass
    for p in ("/root/.axon_site/_ro/trn_rl_repo", "/opt/trn_rl_repo"):
        if os.path.isdir(p) and p not in sys.path:
            sys.path.insert(0, p)
    import concourse.bass  # noqa: F401


_ensure_concourse()

import bass_rust  # noqa: E402
import concourse.bass as bass  # noqa: E402
import concourse.mybir as mybir  # noqa: E402
import concourse.tile as tile  # noqa: E402

F32 = mybir.dt.float32
F32R = mybir.dt.float32r
BF16 = mybir.dt.bfloat16
AF = mybir.ActivationFunctionType

B, S, D = 2, 4096, 768
H, HD = 12, 64
NCORES = 8
HPC = 3            # heads per core
SC = 512           # q-chunk width
KG = 2             # k-tiles per exp slab
SCALE = 1.0 / np.sqrt(HD).astype(np.float32)


def _r(ap):
    return ap.bitcast(F32R)


def split_sp_waits(nc, max_waits=1):
    """This walrus build rejects instructions carrying more than one sync
    wait (setupSyncWait "Too many sync wait commands", any engine struct);
    hoist extras onto preceding single-wait NoOps on the same engine."""
    n = 0
    for f in nc.m.functions:
        for blk in f.blocks:
            out = []
            for inst in blk.instructions:
                si = inst.sync_info
                if si is not None and len(si.on_wait) > max_waits:
                    waits = list(si.on_wait)
                    keep, extra = waits[-max_waits:], waits[:-max_waits]
                    for w in extra:
                        n += 1
                        nop = bass_rust.InstNoOp(name=f"I-wsplit-{n}", ins=[], outs=[])
                        nop.engine = inst.engine
                        nop.sync_info = bass_rust.SyncInfo(on_wait=[w], on_update=[])
                        out.append(nop)
                    inst.sync_info = bass_rust.SyncInfo(
                        on_wait=keep, on_update=list(si.on_update))
                out.append(inst)
            blk.instructions = out
    return n


def build_nc(s=S, with_mask=False, split=True):
    from concourse.masks import make_identity

    nkt = s // 128          # k tiles
    nch = s // SC           # chunks
    dt = D // 128           # 6
    nkg = nkt // KG

    nc = bass.Bass()
    x_d = nc.dram_tensor("x", [s, D], F32, kind="ExternalInput")
    wq_d = nc.dram_tensor("wq", [D, HPC * HD], F32, kind="ExternalInput")
    bq_d = nc.dram_tensor("bq", [HPC * HD], F32, kind="ExternalInput")
    wk_d = nc.dram_tensor("wk", [D, HPC * HD], F32, kind="ExternalInput")
    bk_d = nc.dram_tensor("bk", [HPC * HD], F32, kind="ExternalInput")
    wv_d = nc.dram_tensor("wv", [D, HPC * HD], F32, kind="ExternalInput")
    bv_d = nc.dram_tensor("bv", [HPC * HD], F32, kind="ExternalInput")
    wo_d = nc.dram_tensor("wo", [HPC * HD, D], F32, kind="ExternalInput")
    bo_d = nc.dram_tensor("bo", [D], F32, kind="ExternalInput")
    if with_mask:
        mask_d = nc.dram_tensor("mask", [s], F32, kind="ExternalInput")
    out_d = nc.dram_tensor("out", [s, D], F32, kind="ExternalOutput")

    with tile.TileContext(nc) as tc:
        with (
            tc.tile_pool(name="const", bufs=1) as const,
            tc.tile_pool(name="pers", bufs=1) as pers,
            tc.tile_pool(name="xp", bufs=3) as xp,
            tc.tile_pool(name="xtp", bufs=2) as xtp,
            tc.tile_pool(name="atp", bufs=4) as atp,
            tc.tile_pool(name="nrm", bufs=2) as nrm,
            tc.tile_pool(name="ctxp", bufs=6) as ctxp,
            tc.tile_pool(name="outp", bufs=2) as outp,
            tc.tile_pool(name="ps", bufs=1, space="PSUM") as ps,
        ):
            # ---- constants / weights ----
            ident = const.tile([128, 128], F32, name="ident")
            make_identity(nc, ident)
            ones_st = const.tile([1, 128], F32, name="ones_st")
            nc.gpsimd.memset(ones_st, 1.0)
            ones_f = const.tile([1, 128], F32R, name="ones_f")
            nc.vector.tensor_copy(ones_f, ones_st)
            ones_b = const.tile([1, 128], BF16, name="ones_b")
            nc.gpsimd.memset(ones_b, 1.0)

            # stage fp32 weights then round to f32r via DVE copies
            # (fp32r matmul operands must be produced as float32r)
            wq_sb = const.tile([128, dt, HPC * HD], F32R, name="wq_sb")
            wk_sb = const.tile([128, dt, HPC * HD], F32R, name="wk_sb")
            wv_sb = const.tile([128, dt, 256], F32R, name="wv_sb")
            for (dsrc, dst, pad) in ((wq_d, wq_sb, False), (wk_d, wk_sb, False),
                                     (wv_d, wv_sb, True)):
                if pad:
                    wstp = const.tile([128, dt, 256], F32, name="wstp")
                    nc.gpsimd.memset(wstp, 0.0)
                    nc.sync.dma_start(out=wstp[:, :, 0:HPC * HD],
                                      in_=dsrc[:].rearrange("(t p) h -> p t h", p=128))
                    nc.vector.tensor_copy(dst, wstp)
                else:
                    wst = const.tile([128, dt, HPC * HD], F32, tag="wstage", name="wst")
                    nc.sync.dma_start(out=wst,
                                      in_=dsrc[:].rearrange("(t p) h -> p t h", p=128))
                    nc.vector.tensor_copy(dst, wst)
            bv_st = const.tile([1, 256], F32, name="bv_st")
            nc.gpsimd.memset(bv_st, 0.0)
            nc.sync.dma_start(out=bv_st[:, 0:HPC * HD],
                              in_=bv_d[:].rearrange("(o h) -> o h", o=1))
            bv_pad = const.tile([1, 256], F32R, name="bv_pad")
            nc.vector.tensor_copy(bv_pad, bv_st)

            wo_st = const.tile([64, HPC, D], F32, name="wo_st")
            nc.sync.dma_start(out=wo_st, in_=wo_d[:].rearrange("(h p) d -> p h d", p=64))
            wo_sb = const.tile([64, HPC, D], BF16, name="wo_sb")
            nc.vector.tensor_copy(wo_sb, wo_st)
            bo_st = const.tile([1, D], F32, name="bo_st")
            nc.sync.dma_start(out=bo_st, in_=bo_d[:].rearrange("(o h) -> o h", o=1))
            bo_sb = const.tile([1, D], BF16, name="bo_sb")
            nc.vector.tensor_copy(bo_sb, bo_st)

            bq01 = const.tile([128, 1], F32, name="bq01")
            nc.sync.dma_start(out=bq01, in_=bq_d[0:128].rearrange("(p o) -> p o", o=1))
            bq2 = const.tile([64, 1], F32, name="bq2")
            nc.sync.dma_start(out=bq2, in_=bq_d[128:192].rearrange("(p o) -> p o", o=1))
            bk01 = const.tile([128, 1], F32, name="bk01")
            nc.sync.dma_start(out=bk01, in_=bk_d[0:128].rearrange("(p o) -> p o", o=1))
            bk2 = const.tile([64, 1], F32, name="bk2")
            nc.sync.dma_start(out=bk2, in_=bk_d[128:192].rearrange("(p o) -> p o", o=1))

            if with_mask:
                maskb = const.tile([128, nkt], F32, name="maskb")
                nc.sync.dma_start(out=maskb, in_=mask_d[:].rearrange("(t p) -> p t", p=128))
                # bias = (mask - 1) * 1e6   (0 where mask==1, -1e6 where 0)
                nc.vector.tensor_scalar(maskb, maskb, 1.0, 1e6,
                                        mybir.AluOpType.subtract, mybir.AluOpType.mult)

            # ---- persistent activations ----
            qT01 = pers.tile([128, s], F32R, name="qT01")
            qT2 = pers.tile([64, s], F32R, name="qT2")
            kT01 = pers.tile([128, s], F32R, name="kT01")
            kT2 = pers.tile([64, s], F32R, name="kT2")
            v_all = pers.tile([128, nkt, HPC, HD + 1], BF16, name="v_all")
            nc.gpsimd.memset(v_all[:, :, :, HD:HD + 1], 1.0)

            # ---- stage A: transpose + projections ----
            for ci in range(nch):
                xt = xtp.tile([128, dt, SC], F32R, name="xt")
                for n in range(SC // 128):
                    st = ci * (SC // 128) + n
                    xarr = xp.tile([128, D], F32, name="xarr")
                    nc.sync.dma_start(out=xarr, in_=x_d[st * 128:(st + 1) * 128, :])
                    for d in range(dt):
                        tp = ps.tile([128, 128], F32, tag="acc", bufs=4, name="tp")
                        nc.tensor.transpose(tp, xarr[:, d * 128:(d + 1) * 128], ident)
                        nc.vector.tensor_copy(xt[:, d, n * 128:(n + 1) * 128], tp)
                    # V projection for this s-tile (fp32r, padded N=256)
                    vps = ps.tile([128, 256], F32, tag="big", bufs=2, name="vps")
                    for d in range(dt):
                        nc.tensor.matmul(vps, xt[:, d, n * 128:(n + 1) * 128],
                                         wv_sb[:, d, :],
                                         start=(d == 0), stop=False)
                    nc.tensor.matmul(vps, ones_f, bv_pad, start=False, stop=True)
                    nc.vector.tensor_copy(
                        v_all[:, st, :, 0:HD],
                        vps[:, 0:HPC * HD].rearrange("p (h c) -> p h c", h=HPC))
                # Q/K projections for the chunk
                for wsb, d01, d2, b01, b2 in ((wq_sb, qT01, qT2, bq01, bq2),
                                              (wk_sb, kT01, kT2, bk01, bk2)):
                    # separate PSUM banks per head: start=True zeroes a whole
                    # 2KB bank region, so accumulation groups can't share one.
                    # f32r matmuls can't col-tile (out base must be 0), so
                    # head1's result goes through SBUF + a partition-moving DMA.
                    psa = ps.tile([64, SC], F32, tag="acc", bufs=4, name="psa")
                    psb = ps.tile([64, SC], F32, tag="acc", bufs=4, name="psb")
                    for d in range(dt):
                        nc.tensor.matmul(psa, wsb[:, d, 0:64],
                                         xt[:, d, :], start=(d == 0), stop=(d == dt - 1))
                        nc.tensor.matmul(psb, wsb[:, d, 64:128],
                                         xt[:, d, :], start=(d == 0), stop=(d == dt - 1))
                    nc.vector.tensor_scalar_add(d01[0:64, ci * SC:(ci + 1) * SC],
                                                psa, b01[0:64, :])
                    h1t = xp.tile([64, SC], F32R, tag="h1t", name="h1t")
                    nc.vector.tensor_scalar_add(h1t, psb, b01[64:128, :])
                    nc.sync.dma_start(out=d01[64:128, ci * SC:(ci + 1) * SC], in_=h1t)
                    ps2 = ps.tile([64, SC], F32, tag="acc", bufs=4, name="ps2")
                    for d in range(dt):
                        nc.tensor.matmul(ps2, wsb[:, d, 128:192], xt[:, d, :],
                                         start=(d == 0), stop=(d == dt - 1))
                    nc.vector.tensor_scalar_add(d2[:, ci * SC:(ci + 1) * SC], ps2, b2)

            # ---- stage B/C: attention + output projection, per q-chunk ----
            def attn_stream(j, h, ctx_ps):
                if h < 2:
                    lo, hi = (0, 64) if h == 0 else (64, 128)
                    kT, qT = kT01[lo:hi, :], qT01[lo:hi, :]
                else:
                    kT, qT = kT2, qT2
                for g in range(nkg):
                    sc_ps = ps.tile([128, KG * SC], F32, tag="big", bufs=2, name="sc_ps")
                    for i in range(KG):
                        kt = g * KG + i
                        nc.tensor.matmul(sc_ps[:, i * SC:(i + 1) * SC],
                                         kT[:, kt * 128:(kt + 1) * 128],
                                         qT[:, j * SC:(j + 1) * SC])
                    at = atp.tile([128, KG * SC], BF16, name="at")
                    if with_mask:
                        for i in range(KG):
                            kt = g * KG + i
                            nc.scalar.activation(at[:, i * SC:(i + 1) * SC],
                                                 sc_ps[:, i * SC:(i + 1) * SC],
                                                 AF.Exp, bias=maskb[:, kt:kt + 1],
                                                 scale=float(SCALE))
                    else:
                        nc.scalar.activation(at, sc_ps, AF.Exp, scale=float(SCALE))
                    for i in range(KG):
                        kt = g * KG + i
                        nc.tensor.matmul(ctx_ps, v_all[:, kt, h, :],
                                         at[:, i * SC:(i + 1) * SC],
                                         start=(kt == 0), stop=(kt == nkt - 1))

            for j in range(nch):
                ctx0 = ps.tile([65, SC], F32, tag="acc", bufs=4, name="ctx0")
                ctx1 = ps.tile([65, SC], F32, tag="acc", bufs=4, name="ctx1")
                # heads 0,1 interleaved (row-packed matmuls, ACT alternates)
                if True:
                    kT0, qT0 = kT01[0:64, :], qT01[0:64, :]
                    kT1, qT1 = kT01[64:128, :], qT01[64:128, :]
                    for g in range(nkg):
                        for h, (kT, qT, ctx_ps) in enumerate(
                                ((kT0, qT0, ctx0), (kT1, qT1, ctx1))):
                            sc_ps = ps.tile([128, KG * SC], F32, tag="big", bufs=2,
                                            name="sc_ps")
                            for i in range(KG):
                                kt = g * KG + i
                                nc.tensor.matmul(sc_ps[:, i * SC:(i + 1) * SC],
                                                 kT[:, kt * 128:(kt + 1) * 128],
                                                 qT[:, j * SC:(j + 1) * SC])
                            at = atp.tile([128, KG * SC], BF16, name="at")
                            if with_mask:
                                for i in range(KG):
                                    kt = g * KG + i
                                    nc.scalar.activation(at[:, i * SC:(i + 1) * SC],
                                                         sc_ps[:, i * SC:(i + 1) * SC],
                                                         AF.Exp,
                                                         bias=maskb[:, kt:kt + 1],
                                                         scale=float(SCALE))
                            else:
                                nc.scalar.activation(at, sc_ps, AF.Exp, scale=float(SCALE))
                            for i in range(KG):
                                kt = g * KG + i
                                nc.tensor.matmul(ctx_ps, v_all[:, kt, h, :],
                                                 at[:, i * SC:(i + 1) * SC],
                                                 start=(kt == 0), stop=(kt == nkt - 1))
                ctx2 = ps.tile([65, SC], F32, tag="acc", bufs=4, name="ctx2")
                attn_stream(j, 2, ctx2)

                # normalize: ctx[0:64] / ctx[64]
                ctx_sb = []
                for h, ctx_ps in ((0, ctx0), (1, ctx1), (2, ctx2)):
                    rs = nrm.tile([1, SC], F32R, name="rs")
                    with nc.allow_low_precision(reason="f32r recip feeds PE broadcast"):
                        nc.vector.reciprocal(rs, ctx_ps[64:65, :])
                    rb_ps = ps.tile([64, SC], F32, tag="acc", bufs=4, name="rb_ps")
                    nc.tensor.matmul(rb_ps, ones_f[0:1, 0:64], rs)
                    rb = nrm.tile([64, SC], F32, name="rb")
                    nc.vector.tensor_copy(rb, rb_ps)
                    cs = ctxp.tile([64, SC], BF16, name="cs")
                    nc.vector.tensor_mul(cs, ctx_ps[0:64, :], rb)
                    ctx_sb.append(cs)

                # output projection for this chunk's 4 s-tiles
                for n in range(SC // 128):
                    st = j * (SC // 128) + n
                    osb = outp.tile([128, D], F32, name="osb")
                    for c0, c1 in ((0, 512), (512, D)):
                        ops = ps.tile([128, c1 - c0], F32, tag="acc", bufs=4, name="ops")
                        for h in range(HPC):
                            nc.tensor.matmul(ops, ctx_sb[h][:, n * 128:(n + 1) * 128],
                                             wo_sb[:, h, c0:c1],
                                             start=(h == 0), stop=False)
                        nc.tensor.matmul(ops, ones_b, bo_sb[:, c0:c1],
                                         start=False, stop=True)
                        nc.vector.tensor_copy(osb[:, c0:c1], ops)
                    nc.sync.dma_start(out=out_d[st * 128:(st + 1) * 128, :], in_=osb)

    if split:
        split_sp_waits(nc)
    return nc


_BUILD_CACHE = {}


def _get_nc(s, with_mask):
    key = (s, with_mask)
    if key not in _BUILD_CACHE:
        _BUILD_CACHE[key] = build_nc(s, with_mask)
    return _BUILD_CACHE[key]


def make_in_maps(X, mask, Wq, bq, Wk, bk, Wv, bv, Wo, bo, with_mask):
    in_maps = []
    for c in range(NCORES):
        b, hg = divmod(c, 4)
        hsl = slice(HPC * HD * hg, HPC * HD * (hg + 1))
        m = {
            "x": np.ascontiguousarray(X[b]),
            "wq": np.ascontiguousarray(Wq[:, hsl]),
            "bq": np.ascontiguousarray(bq[hsl]),
            "wk": np.ascontiguousarray(Wk[:, hsl]),
            "bk": np.ascontiguousarray(bk[hsl]),
            "wv": np.ascontiguousarray(Wv[:, hsl]),
            "bv": np.ascontiguousarray(bv[hsl]),
            "wo": np.ascontiguousarray(Wo[hsl, :]),
            "bo": np.ascontiguousarray(bo if hg == 0 else np.zeros_like(bo)),
        }
        if with_mask:
            m["mask"] = np.ascontiguousarray(mask[b])
        in_maps.append(m)
    return in_maps


def kernel(X, mask, Wq, bq, Wk, bk, Wv, bv, Wo, bo):
    from concourse.bass_utils import run_bass_kernel_spmd

    X = np.asarray(X, dtype=np.float32)
    mask = np.asarray(mask, dtype=np.float32)
    Wq, bq = np.asarray(Wq, np.float32), np.asarray(bq, np.float32)
    Wk, bk = np.asarray(Wk, np.float32), np.asarray(bk, np.float32)
    Wv, bv = np.asarray(Wv, np.float32), np.asarray(bv, np.float32)
    Wo, bo = np.asarray(Wo, np.float32), np.asarray(bo, np.float32)

    with_mask = not np.all(mask == 1.0)
    nc = _get_nc(S, with_mask)
    in_maps = make_in_maps(X, mask, Wq, bq, Wk, bk, Wv, bv, Wo, bo, with_mask)
    res = run_bass_kernel_spmd(nc, in_maps, list(range(NCORES))).results
    out = np.zeros((B, S, D), dtype=np.float32)
    for c in range(NCORES):
        out[c // 4] += res[c]["out"]
    return out
